# revision 1
# baseline (speedup 1.0000x reference)
"""Trainium2 Bass kernel for MiniMemory: gated linear recurrence.

    mass  = sigmoid(x @ w_mass)            # [B, T]
    decay = sigmoid(x @ w_decay)           # [B, T]
    s_t   = decay_t * s_{t-1} + mass_t * x_t   (elementwise over D)
    out   = s                              # [B, T, D]

Data-parallel over B across 8 NeuronCores (1 sample/core). The decay is a
scalar per timestep, so a 128-step block solves in closed form as a
lower-triangular matmul:

    out_blk = A @ (mass * x_blk) + e * s_carry
    A[t,t'] = prod_{k=t'+1..t} decay_k   (lower-tri, diag=1)
    e[t]    = prod_{k=0..t} decay_k

A^T (the stationary operand) is built exactly with one vector-engine
tensor_tensor_scan: AT[:,f] = d_f * AT[:,f-1] + I[:,f]. Gate logits come
from the tensor engine: PE-transpose x chunks, then 16 accumulating
[K=128,M=2]x[K=128,N=128] matmuls against packed (w_mass|w_decay) columns.
The inter-block carry enters as a rank-1 (K=1) accumulating matmul.
All math fp32; matches the reference to ~1e-6.
"""

import numpy as np


def _ensure_path():
    try:
        import concourse.bass_utils  # noqa: F401
    except ImportError:
        import sys
        for p in ("/opt/trn_rl_repo", "/root/.axon_site/_ro/trn_rl_repo"):
            if p not in sys.path:
                sys.path.insert(0, p)
        import concourse.bass_utils  # noqa: F401


_ensure_path()

import concourse.bacc as bacc  # noqa: E402
import concourse.tile as tile  # noqa: E402
from concourse import mybir  # noqa: E402
from concourse.bass_utils import run_bass_kernel_spmd  # noqa: E402
from concourse.masks import make_identity  # noqa: E402

B, T, D = 8, 4096, 2048
L = 128          # timesteps per block (= partition count)
NCHUNK = D // 128
NCORES = 8
F32 = mybir.dt.float32
AF = mybir.ActivationFunctionType
ALU = mybir.AluOpType


def build_kernel(t_len=T, reps=1):
    nc = bacc.Bacc("TRN2", target_bir_lowering=False, debug=False)
    x_d = nc.dram_tensor("x", [t_len, D], F32, kind="ExternalInput").ap()
    # packed gate weights: w2[p, 2c+j] = (w_mass if j==0 else w_decay)[128c+p]
    w2_d = nc.dram_tensor("w2", [128, 2 * NCHUNK], F32, kind="ExternalInput").ap()
    out_d = nc.dram_tensor("out", [t_len, D], F32, kind="ExternalOutput").ap()

    nblk = t_len // L
    with tile.TileContext(nc) as tc:
        with (
            tc.tile_pool(name="consts", bufs=1) as consts,
            tc.tile_pool(name="xp", bufs=3) as xp,
            tc.tile_pool(name="xtp", bufs=2) as xtp,
            tc.tile_pool(name="wp", bufs=2) as wp,
            tc.tile_pool(name="op", bufs=3) as op,
            tc.tile_pool(name="small", bufs=3) as small,
            tc.tile_pool(name="psO", bufs=4, space="PSUM") as psO,
            tc.tile_pool(name="psT", bufs=2, space="PSUM") as psT,
            tc.tile_pool(name="psS", bufs=2, space="PSUM") as psS,
        ):
            ident = consts.tile([128, 128], F32)
            make_identity(nc, ident)
            ones_row = consts.tile([1, 128], F32)
            nc.vector.memset(ones_row, 1.0)
            w2 = consts.tile([128, 2 * NCHUNK], F32)
            nc.sync.dma_start(out=w2, in_=w2_d)

            for _ in range(reps):
                prev_out = None
                for b in range(nblk):
                    x_sb = xp.tile([128, D], F32, tag="x")
                    nc.sync.dma_start(out=x_sb, in_=x_d[b * L:(b + 1) * L, :])

                    # xT chunks via PE transpose (4 pieces of [128, 512])
                    xT = xtp.tile([128, D], F32, tag="xT")
                    for j in range(4):
                        pT = psT.tile([128, 512], F32, tag="psT")
                        for k in range(4):
                            c = 4 * j + k
                            nc.tensor.transpose(
                                out=pT[:, k * 128:(k + 1) * 128],
                                in_=x_sb[:, c * 128:(c + 1) * 128],
                                identity=ident)
                        sl = slice(j * 512, (j + 1) * 512)
                        if j % 2 == 0:
                            nc.scalar.activation(out=xT[:, sl], in_=pT,
                                                 func=AF.Copy)
                        else:
                            nc.vector.tensor_copy(out=xT[:, sl], in_=pT)

                    # gate logits [2, 128]: accumulate over 16 D-chunks
                    gps = psS.tile([2, 128], F32, tag="pss")
                    for c in range(NCHUNK):
                        nc.tensor.matmul(
                            gps, lhsT=w2[:, 2 * c:2 * c + 2],
                            rhs=xT[:, c * 128:(c + 1) * 128],
                            start=(c == 0), stop=(c == NCHUNK - 1))
                    # row 0 = decay, row 1 = mass (decay row is used in row
                    # form; engine operands must start at partition 0)
                    gsig = small.tile([2, 128], F32, tag="gsig")
                    nc.scalar.activation(out=gsig, in_=gps, func=AF.Sigmoid)
                    drow = gsig[0:1, :]
                    # both gates as columns via one K=2 transpose matmul
                    gcols_ps = psS.tile([128, 2], F32, tag="pss")
                    nc.tensor.matmul(gcols_ps, lhsT=gsig,
                                     rhs=ident[0:2, 0:2],
                                     start=True, stop=True)
                    gcols = small.tile([128, 2], F32, tag="gcols")
                    nc.scalar.activation(out=gcols, in_=gcols_ps, func=AF.Copy)
                    mcol = gcols[:, 1:2]

                    # weighted input w = mass * x
                    w_sb = wp.tile([128, D], F32, tag="w")
                    nc.vector.tensor_scalar_mul(w_sb, x_sb, mcol)

                    # decay broadcast down 128 partitions, then the scan
                    dbc_ps = psS.tile([128, 128], F32, tag="pss")
                    nc.tensor.matmul(dbc_ps, lhsT=ones_row, rhs=drow,
                                     start=True, stop=True)
                    AT = small.tile([128, 128], F32, tag="AT")
                    nc.vector.tensor_tensor_scan(
                        out=AT, data0=dbc_ps, data1=ident, initial=0.0,
                        op0=ALU.mult, op1=ALU.add)

                    # e[t] = prod_{k=0..t} d_k = d_0 * AT[0, t]
                    e_row = small.tile([1, 128], F32, tag="erow")
                    nc.vector.tensor_scalar_mul(e_row, AT[0:1, :],
                                                gsig[0:1, 0:1])

                    carry = None
                    if b > 0:
                        # move last row of prev block to partition 0
                        carry = small.tile([1, D], F32, tag="carry")
                        nc.sync.dma_start(out=carry, in_=prev_out[127:128, :])

                    out_sb = op.tile([128, D], F32, tag="o")
                    for j in range(4):
                        sl = slice(j * 512, (j + 1) * 512)
                        ops = psO.tile([128, 512], F32, tag="psO")
                        nc.tensor.matmul(ops, lhsT=AT, rhs=w_sb[:, sl],
                                         start=True, stop=(b == 0))
                        if b > 0:
                            nc.tensor.matmul(ops, lhsT=e_row,
                                             rhs=carry[0:1, sl],
                                             start=False, stop=True)
                        if j % 2 == 0:
                            nc.scalar.activation(out=out_sb[:, sl], in_=ops,
                                                 func=AF.Copy)
                        else:
                            nc.vector.tensor_copy(out=out_sb[:, sl], in_=ops)
                    nc.sync.dma_start(out=out_d[b * L:(b + 1) * L, :],
                                      in_=out_sb)
                    prev_out = out_sb
    nc.compile()
    return nc


def pack_w2(w_mass, w_decay):
    w2 = np.empty((128, 2 * NCHUNK), dtype=np.float32)
    wm = np.asarray(w_mass, np.float32).reshape(NCHUNK, 128)
    wd = np.asarray(w_decay, np.float32).reshape(NCHUNK, 128)
    w2[:, 0::2] = wd.T
    w2[:, 1::2] = wm.T
    return np.ascontiguousarray(w2)


_CACHE = {}


def _get_nc():
    if "nc" not in _CACHE:
        _CACHE["nc"] = build_kernel(T)
    return _CACHE["nc"]


def kernel(x, w_mass, w_decay):
    x = np.ascontiguousarray(x, dtype=np.float32)
    w2 = pack_w2(w_mass, w_decay)
    nc = _get_nc()
    in_maps = [{"x": x[i], "w2": w2} for i in range(B)]
    res = run_bass_kernel_spmd(nc, in_maps, core_ids=list(range(NCORES)))
    return np.stack([res.results[i]["out"] for i in range(B)], axis=0)



# revision 2
# speedup vs baseline: 30541.1753x; 30541.1753x over previous
"""Trainium2 Bass kernel for MiniMemory: gated linear recurrence.

    mass  = sigmoid(x @ w_mass)            # [B, T]
    decay = sigmoid(x @ w_decay)           # [B, T]
    s_t   = decay_t * s_{t-1} + mass_t * x_t   (elementwise over D)
    out   = s                              # [B, T, D]

Data-parallel over B across 8 NeuronCores (1 sample/core). Per 128-step
block the recurrence solves in closed form as one matmul: with
A[t,t'] = prod_{k=t'+1..t} decay_k (lower-tri) and e[t] = prod_{0..t},

    out_blk = (A * mass^T) @ x_blk + e * s_carry

The mass gate is folded into the tiny [128,128] A matrix, so the matmul
rhs is x itself. The emission is software-pipelined: the gate stage of
block k+1 (DMA, fused multiply-reduce logits, sigmoid, decay row
broadcast) is interleaved with the matmul stage of block k so no engine
queue stalls head-of-line on the serial gate->scan->A2T->matmul chain.

Engines: DVE: affine_mul_reduce logits x2, A^T scan, e row. ACT: A2T =
AT*mass (f32r), sigmoid, PSUM->bf16 out copies. PE: block matmul (f32r),
bf16 rank-1 carry, two tiny bf16 helper matmuls. Pool: idle. DMA: x in
fp32, out bf16 (host upcasts), carry row 127 -> partition 0.
"""

import numpy as np


def _ensure_path():
    try:
        import concourse.bass_utils  # noqa: F401
    except ImportError:
        import sys
        for p in ("/opt/trn_rl_repo", "/root/.axon_site/_ro/trn_rl_repo"):
            if p not in sys.path:
                sys.path.insert(0, p)
        import concourse.bass_utils  # noqa: F401


_ensure_path()

import concourse.bacc as bacc  # noqa: E402
import concourse.tile as tile  # noqa: E402
from concourse import mybir  # noqa: E402
from concourse.bass_utils import run_bass_kernel_spmd  # noqa: E402
from concourse.masks import make_identity  # noqa: E402

B, T, D = 8, 4096, 2048
L = 128          # timesteps per block (= partition count)
NCORES = 8
F32 = mybir.dt.float32
F32R = mybir.dt.float32r
BF16 = mybir.dt.bfloat16
AF = mybir.ActivationFunctionType
ALU = mybir.AluOpType


def build_kernel(t_len=T, reps=1):
    nc = bacc.Bacc("TRN2", target_bir_lowering=False, debug=False)
    x_d = nc.dram_tensor("x", [t_len, D], F32R, kind="ExternalInput").ap()
    wm_d = nc.dram_tensor("wm_bc", [128, D], F32, kind="ExternalInput").ap()
    wd_d = nc.dram_tensor("wd_bc", [128, D], F32, kind="ExternalInput").ap()
    out_d = nc.dram_tensor("out", [t_len, D], BF16, kind="ExternalOutput").ap()

    nblk = t_len // L
    with tile.TileContext(nc) as tc:
        with (
            tc.tile_pool(name="consts", bufs=1) as consts,
            tc.tile_pool(name="xp", bufs=4) as xp,
            tc.tile_pool(name="op", bufs=3) as op,
            tc.tile_pool(name="jkv", bufs=1) as jkv,
            tc.tile_pool(name="tiny", bufs=6) as tiny,
            tc.tile_pool(name="atp", bufs=2) as atp,
            tc.tile_pool(name="a2p", bufs=2) as a2p,
            tc.tile_pool(name="psO", bufs=3, space="PSUM") as psO,
            tc.tile_pool(name="psS", bufs=2, space="PSUM") as psS,
        ):
            ident = consts.tile([128, 128], F32)
            make_identity(nc, ident)
            ident_bf = consts.tile([128, 128], BF16)
            nc.scalar.activation(out=ident_bf, in_=ident, func=AF.Copy)
            ones_row = consts.tile([1, 128], BF16)
            nc.vector.memset(ones_row, 1.0)
            wmb = consts.tile([128, D], F32)
            nc.sync.dma_start(out=wmb, in_=wm_d)
            wdb = consts.tile([128, D], F32)
            nc.sync.dma_start(out=wdb, in_=wd_d)
            junk_v = jkv.tile([128, D], BF16)

            for _ in range(reps):
                st = {}

                def g_head(b):
                    s = st[b] = {}
                    s["x"] = xp.tile([128, D], F32R, tag="x", name="x_sb")
                    nc.sync.dma_start(out=s["x"],
                                      in_=x_d[b * L:(b + 1) * L, :])
                    x32 = s["x"].bitcast(F32)
                    s["glog"] = tiny.tile([128, 2], F32, tag="glog", name="glog")
                    nc.vector.affine_mul_reduce(
                        out=junk_v, accum_out=s["glog"][:, 0:1], in0=x32,
                        in1=wmb, scale=1.0, bias=0.0)
                    nc.vector.affine_mul_reduce(
                        out=junk_v, accum_out=s["glog"][:, 1:2], in0=x32,
                        in1=wdb, scale=1.0, bias=0.0)

                def g_tail(b):
                    s = st[b]
                    s["gsig"] = tiny.tile([128, 2], F32, tag="gsig", name="gsig")
                    nc.scalar.activation(out=s["gsig"], in_=s["glog"],
                                         func=AF.Sigmoid)
                    s["gb"] = tiny.tile([128, 2], BF16, tag="gb", name="gb")
                    nc.scalar.activation(out=s["gb"], in_=s["gsig"],
                                         func=AF.Copy)
                    drow_ps = psS.tile([1, 128], F32, tag="pss")
                    nc.tensor.matmul(drow_ps, lhsT=s["gb"][:, 1:2],
                                     rhs=ident_bf, start=True, stop=True)
                    s["drow"] = tiny.tile([1, 128], BF16, tag="drow", name="drow")
                    nc.scalar.activation(out=s["drow"], in_=drow_ps,
                                         func=AF.Copy)
                    s["dbc"] = psS.tile([128, 128], F32, tag="pss", name="dbc_ps")
                    nc.tensor.matmul(s["dbc"], lhsT=ones_row, rhs=s["drow"],
                                     start=True, stop=True)

                def m_stage(b, prev_carry):
                    s = st[b]
                    AT = atp.tile([128, 128], F32, tag="AT")
                    nc.vector.tensor_tensor_scan(
                        out=AT, data0=s["dbc"], data1=ident, initial=0.0,
                        op0=ALU.mult, op1=ALU.add)
                    # reversed column order: out partition p = timestep
                    # 127-p, so the block's final state lands at partition 0
                    A2T = a2p.tile([128, 128], F32R, tag="A2T")
                    nc.scalar.activation(out=A2T[:, ::-1], in_=AT,
                                         func=AF.Copy,
                                         scale=s["gsig"][:, 0:1])
                    if b > 0:
                        e_row = tiny.tile([1, 128], BF16, tag="erow")
                        nc.vector.tensor_scalar_mul(e_row[0:1, ::-1],
                                                    AT[0:1, :],
                                                    s["gsig"][0:1, 1:2])
                    out_sb = op.tile([128, D], BF16, tag="o")
                    opst = [psO.tile([128, 1024], F32, tag="psO",
                                     name="ops") for _ in range(2)]
                    for h in range(2):
                        for q in range(2):
                            sl = slice(h * 1024 + q * 512,
                                       h * 1024 + (q + 1) * 512)
                            nc.tensor.matmul(
                                opst[h][:, q * 512:(q + 1) * 512],
                                lhsT=A2T, rhs=s["x"][:, sl],
                                start=True, stop=(b == 0),
                                skip_group_check=True)
                    for h in range(2):
                        if b > 0:
                            for q in range(2):
                                sl = slice(h * 1024 + q * 512,
                                           h * 1024 + (q + 1) * 512)
                                nc.tensor.matmul(
                                    opst[h][:, q * 512:(q + 1) * 512],
                                    lhsT=e_row,
                                    rhs=prev_carry[0:1, sl],
                                    start=False, stop=True,
                                    skip_group_check=True)
                        nc.scalar.activation(
                            out=out_sb[:, h * 1024:(h + 1) * 1024],
                            in_=opst[h], func=AF.Copy)
                    nc.sync.dma_start(out=out_d[b * L:(b + 1) * L, :],
                                      in_=out_sb)
                    del st[b]
                    return out_sb

                prev_carry = None
                g_head(0)
                g_head(1)
                g_tail(0)
                for b in range(nblk):
                    if b + 2 < nblk:
                        g_head(b + 2)
                    if b + 1 < nblk:
                        g_tail(b + 1)
                    carry = m_stage(b, prev_carry)
                    prev_carry = carry
    nc.compile()
    return nc


_CACHE = {}


def _get_nc():
    if "nc" not in _CACHE:
        _CACHE["nc"] = build_kernel(T)
    return _CACHE["nc"]


def kernel(x, w_mass, w_decay):
    x = np.ascontiguousarray(x, dtype=np.float32)
    wm_bc = np.ascontiguousarray(
        np.broadcast_to(np.asarray(w_mass, np.float32)[None, :], (128, D)))
    wd_bc = np.ascontiguousarray(
        np.broadcast_to(np.asarray(w_decay, np.float32)[None, :], (128, D)))
    nc = _get_nc()
    in_maps = [{"x": x[i], "wm_bc": wm_bc, "wd_bc": wd_bc} for i in range(B)]
    res = run_bass_kernel_spmd(nc, in_maps, core_ids=list(range(NCORES)))
    out = np.stack(
        [np.asarray(res.results[i]["out"]).astype(np.float32)
         for i in range(B)], axis=0)
    # on-chip row order within each 128-block is reversed
    return np.ascontiguousarray(
        out.reshape(B, T // L, L, D)[:, :, ::-1, :].reshape(B, T, D))
